# revision 11
# baseline (speedup 1.0000x reference)
"""Trainium2 Bass kernel for nn_Capsule: capsule routing head.

Math: the einsum 'nco,pbo->bno' factorizes as xp[b,n,o] = W[n,o] * X[b,o]
with W = caps_weights.sum(c) (64x128) and X = x.sum(p) (256x128), so the
kernel is a memory-bound reduction of x (151 MB total, 18.9 MB/core)
followed by a tiny per-batch routing loop.

Sharding: data-parallel over batch (dim 1 of x), 32 batch elements per
core; caps_weights replicated; no cross-core communication.

Per-core pipeline (v2 — DMA-roofline design):
  - 9 p-slabs of x (128, 4096) stream in, each split in half across the
    two HWDGE rings (sync + scalar) so slabs COMPLETE sequentially every
    ~5us instead of in pairs. caps_weights rides the gpsimd SWDGE queue
    so the x stream owns both HWDGE rings from the first descriptor.
  - p-reduction is restructured away from the old per-batch one-hot
    matmuls (which kept the PE 86% busy -> HAM 50% duty-cycle throttle
    + 16us of LDWEIGHTS):
      * DVE + Pool engines pre-accumulate slabs 0-3 into accA and 4-7
        into accB (half-slab per engine, ~20%/17% duty).
      * PE reduces each acc with an ALL-ONES (128,1) stationary: psum
        chunk c (1,512) accumulates A, B, and raw slab-8 passes
        (3 matmuls per chunk, moving free 512 -> full fp32r rate).
        PE duty ~15% -> no HAM throttle; stationary never changes.
      * slab 7 adds and slab 8 DMA+matmuls are chunk-split so the tail
        after the last HBM byte is ~2us (chunk mm -> copy -> rearrange).
  - X lands flat (1, 4096) in psum chunks; copies (DVE/ACT alternating)
    build xflat, one SBUF->SBUF DMA scatters it to (32, 128).
  - Routing in b-on-partitions layout; sqrt(q) = Exp(0.5*Ln(q)) keeps
    every ACT function in one table (pinned); the delta path
    (M2=X*X -> MS=M2*S -> transpose -> matmul) runs on Pool+PE in
    parallel with the DVE/ACT norm chain; softmax normalization is
    deferred through rsum and folded into Ln via its scale argument.
"""

import numpy as np

# ---- problem constants (hardcoded per contract) ----
P_TOT = 1152
BATCH = 256
O = 128
N_CAPS = 64
CAPS_DIM = 16
ITERATIONS = 3
N_CORES = 8
B_LOC = BATCH // N_CORES          # 32 batch elements per core
PT = P_TOT // 128                 # 9 p-slabs
FLAT = B_LOC * O                  # 4096 flat (b,o) elements
CH = 512                          # psum chunk (max fp32 free per bank)
NCH = FLAT // CH                  # 8 chunks
HALF = FLAT // 2

_cache = {}


def _pin_act_table():
    """Force every ACT function onto the one table containing
    Exp+Ln+Square+Copy, so the kernel needs a single ACT_TABLE_LOAD."""
    import functools
    import concourse.hw_specs as hw_specs
    import concourse.bacc as bacc_mod

    if getattr(hw_specs.get_activation_tables, "_capsule_pinned", False):
        return
    orig = hw_specs.get_activation_tables

    @functools.cache
    def pinned(module_arch):
        tabs = orig(module_arch)
        keep = None
        for name, fns in tabs.items():
            names = {f.name for f in fns}
            if {"Exp", "Ln", "Square", "Copy", "Identity"} <= names:
                keep = name
                break
        if keep is None:
            return tabs
        return {n: (fns if n == keep else type(fns)()) for n, fns in tabs.items()}

    pinned._capsule_pinned = True
    hw_specs.get_activation_tables = pinned
    bacc_mod.get_activation_tables = pinned


def _build(n_xq=2, stage="full"):
    """n_xq: number of DMA queues carrying the x stream.
    2 = sync + scalar HWDGE rings; 3 = + gpsimd SWDGE queue."""
    _pin_act_table()
    import concourse.bacc as bacc
    import concourse.tile as tile
    import concourse.mybir as mybir
    from concourse.masks import make_identity

    f32 = mybir.dt.float32
    f32r = mybir.dt.float32r
    AF = mybir.ActivationFunctionType
    AX = mybir.AxisListType
    OP = mybir.AluOpType

    nc = bacc.Bacc(None, target_bir_lowering=False)

    x_in = nc.dram_tensor("x", [P_TOT, B_LOC, O], f32r, kind="ExternalInput")
    w_in = nc.dram_tensor("caps_weights", [N_CAPS, CAPS_DIM, O], f32,
                          kind="ExternalInput")
    ones_in = nc.dram_tensor("ones", [128, 1], f32r, kind="ExternalInput")
    out_d = nc.dram_tensor("out", [B_LOC, O], f32, kind="ExternalOutput")
    # DRAM bounce for the flat->(b,o) rearrange: SBUF->SBUF cross-partition
    # scatter is not expressible as DGE descriptors (verified wrong on HW)
    xsc = nc.dram_tensor("xscratch", [B_LOC, O], f32, kind="Internal")

    xv = x_in.rearrange("(t p) b o -> t p (b o)", p=128)  # (9, 128, 4096)

    with tile.TileContext(nc) as tc:
        with (
            tc.tile_pool(name="xin", bufs=1) as xpool,
            tc.tile_pool(name="wrk", bufs=1) as wrk,
            tc.tile_pool(name="ps", bufs=8, space="PSUM") as ps,
        ):
            # ---- destination tiles for the stream ----
            accA = xpool.tile([128, FLAT], f32r, tag="accA")
            accB = xpool.tile([128, FLAT], f32r, tag="accB")
            slabs = {}
            slot_of = {1: 0, 2: 1, 3: 2, 5: 3, 6: 0, 7: 1, 8: 2}
            sl_tiles = [xpool.tile([128, FLAT], f32r, tag="slab", bufs=4,
                                   name=f"sl{i}") for i in range(4)]

            def dest(t):
                if t == 0:
                    return accA
                if t == 4:
                    return accB
                return sl_tiles[slot_of[t]]

            # ---- DMA helpers ----
            # Tile deps are PROGRAM-ORDER based: an add reading a slab buffer
            # binds to the most recent DMA into that buffer emitted BEFORE it.
            # So DMA(t) and add(t) must be emitted interleaved per slab.
            if n_xq == 2:
                def emit_dma(t):
                    d = dest(t)
                    if t == 8:
                        for c in range(NCH):
                            eng = [nc.sync, nc.scalar][c % 2]
                            eng.dma_start(d[:, c * CH:(c + 1) * CH],
                                          xv[8][:, c * CH:(c + 1) * CH])
                    else:
                        nc.sync.dma_start(d[:, 0:HALF], xv[t][:, 0:HALF])
                        nc.scalar.dma_start(d[:, HALF:FLAT],
                                            xv[t][:, HALF:FLAT])
            else:
                cuts = [0, 1376, 2752, FLAT]

                def emit_dma(t):
                    d = dest(t)
                    if t == 8:
                        for c in range(NCH):
                            eng = [nc.sync, nc.scalar, nc.gpsimd][c % 3]
                            eng.dma_start(d[:, c * CH:(c + 1) * CH],
                                          xv[8][:, c * CH:(c + 1) * CH])
                    else:
                        for qi, eng in enumerate([nc.sync, nc.scalar,
                                                  nc.gpsimd]):
                            eng.dma_start(d[:, cuts[qi]:cuts[qi + 1]],
                                          xv[t][:, cuts[qi]:cuts[qi + 1]])

            def emit_add(acc, t, chunked=False):
                s = dest(t)
                if chunked:
                    for c in range(NCH):
                        eng = nc.vector if c % 2 == 0 else nc.gpsimd
                        cs, ce = c * CH, (c + 1) * CH
                        eng.tensor_tensor(acc[:, cs:ce], acc[:, cs:ce],
                                          s[:, cs:ce], OP.add)
                else:
                    nc.vector.tensor_tensor(acc[:, 0:HALF], acc[:, 0:HALF],
                                            s[:, 0:HALF], OP.add)
                    nc.gpsimd.tensor_tensor(acc[:, HALF:FLAT],
                                            acc[:, HALF:FLAT],
                                            s[:, HALF:FLAT], OP.add)

            # ---- stream prologue: w on SWDGE, then slab 0 ----
            w_sb = wrk.tile([N_CAPS, CAPS_DIM * O], f32)
            if n_xq == 2:
                nc.gpsimd.dma_start(w_sb[:], w_in.rearrange("n c o -> n (c o)"))
                emit_dma(0)
            else:
                emit_dma(0)   # keep the SWDGE queue x-first when it carries x
                nc.gpsimd.dma_start(w_sb[:], w_in.rearrange("n c o -> n (c o)"))
            emit_dma(1)

            # ---- constants ----
            ident = wrk.tile([128, 128], f32)
            make_identity(nc, ident[:])
            ones_st = wrk.tile([128, 1], f32r)
            nc.gpsimd.dma_start(ones_st[:], ones_in[:])
            ones32 = wrk.tile([B_LOC, 1], f32)
            nc.vector.memset(ones32[:], 1.0)
            unif = wrk.tile([N_CAPS, B_LOC], f32)
            nc.vector.memset(unif[:], 1.0 / N_CAPS)

            # ---- capsule weight prep (overlaps the x stream) ----
            t1 = wrk.tile([N_CAPS, 8 * O], f32)
            nc.vector.tensor_tensor(t1[:], w_sb[:, :8 * O], w_sb[:, 8 * O:], OP.add)
            t2 = wrk.tile([N_CAPS, 4 * O], f32)
            nc.vector.tensor_tensor(t2[:], t1[:, :4 * O], t1[:, 4 * O:], OP.add)
            t3 = wrk.tile([N_CAPS, 2 * O], f32)
            nc.vector.tensor_tensor(t3[:], t2[:, :2 * O], t2[:, 2 * O:], OP.add)
            w_no = wrk.tile([N_CAPS, O], f32)          # W[n,o]
            nc.vector.tensor_tensor(w_no[:], t3[:, :O], t3[:, O:], OP.add)

            ps_wt = ps.tile([O, N_CAPS], f32, tag="ps")
            nc.tensor.transpose(ps_wt[:], w_no[:], ident[:N_CAPS, :N_CAPS])
            wt_on = wrk.tile([O, N_CAPS], f32)          # W^T[o,n]
            nc.vector.tensor_copy(wt_on[:], ps_wt[:])
            ps_s0 = ps.tile([B_LOC, O], f32, tag="ps", name="ps_s0")
            nc.tensor.matmul(ps_s0[:], unif[:], w_no[:], start=True, stop=True)
            s0_sb = wrk.tile([B_LOC, O], f32)           # S0 = (1/64) sum_n W
            nc.vector.tensor_copy(s0_sb[:], ps_s0[:])

            # ---- stream body: DMA(t) then add(t), interleaved ----
            emit_add(accA, 1)
            emit_dma(2)
            emit_add(accA, 2)
            emit_dma(3)
            emit_add(accA, 3)
            emit_dma(4)
            emit_dma(5)
            emit_add(accB, 5)
            emit_dma(6)
            emit_add(accB, 6)
            emit_dma(7)
            # slab 7 adds chunk-split so the B matmul pass can start while
            # slab 7 is still arriving
            emit_add(accB, 7, chunked=True)
            emit_dma(8)

            # ---- PE reduction: 3 passes x 8 chunks, all-ones stationary ----
            ps_ch = [ps.tile([1, CH], f32, tag="ps", name=f"ps_ch{c}")
                     for c in range(NCH)]
            s8 = dest(8)
            for c in range(NCH):
                cs, ce = c * CH, (c + 1) * CH
                nc.tensor.matmul(ps_ch[c][:], ones_st[:], accA[:, cs:ce],
                                 start=True, stop=False, skip_group_check=True)
            for c in range(NCH):
                cs, ce = c * CH, (c + 1) * CH
                nc.tensor.matmul(ps_ch[c][:], ones_st[:], accB[:, cs:ce],
                                 start=False, stop=False, skip_group_check=True)
            xflat = wrk.tile([1, FLAT], f32)
            for c in range(NCH):
                cs, ce = c * CH, (c + 1) * CH
                nc.tensor.matmul(ps_ch[c][:], ones_st[:], s8[:, cs:ce],
                                 start=False, stop=True, skip_group_check=True)
                if c % 2 == 0:
                    nc.vector.tensor_copy(xflat[:, cs:ce], ps_ch[c][:])
                else:
                    nc.scalar.copy(xflat[:, cs:ce], ps_ch[c][:])

            # ---- rearrange flat (1,(b o)) -> (b,o) via DRAM bounce ----
            x32 = wrk.tile([B_LOC, O], f32)
            nc.sync.dma_start(xsc.rearrange("b o -> (b o)")[None, :], xflat[:])
            nc.sync.dma_start(x32[:], xsc[:])

            if stage == "x32":
                o_dbg = wrk.tile([B_LOC, O], f32)
                nc.vector.tensor_scalar_add(o_dbg[:], x32[:], 0.0)
                nc.sync.dma_start(out_d[:], o_dbg[:])

            # ---- routing (b on partitions) ----
            M2 = wrk.tile([B_LOC, O], f32)
            nc.gpsimd.tensor_tensor(M2[:], x32[:], x32[:], OP.mult)

            iters = ITERATIONS if stage == "full" else 0
            sq = wrk.tile([B_LOC, O], f32)
            lg = None
            rsum = None
            r2 = None
            exT = None
            for it in range(iters):
                if it == 0:
                    s_ap = s0_sb[:]
                else:
                    ps_s = ps.tile([B_LOC, O], f32, tag="ps", name=f"ps_s{it}")
                    nc.tensor.matmul(ps_s[:], exT[:], w_no[:],
                                     start=True, stop=True)
                    s_ap = ps_s[:]
                # norm chain (DVE + ACT)
                xs = wrk.tile([B_LOC, O], f32, tag="xs", name=f"xs{it}")
                nc.vector.tensor_tensor(xs[:], x32[:], s_ap, OP.mult)
                qpre = wrk.tile([B_LOC, 1], f32, tag="qpre", name=f"qpre{it}")
                # (tensor_tensor_reduce crashes the exec unit on this runtime
                #  -> NRT_EXEC_UNIT_UNRECOVERABLE; use mult + reduce instead)
                nc.vector.tensor_tensor(sq[:], xs[:], xs[:], OP.mult)
                nc.vector.tensor_reduce(qpre[:], sq[:], AX.X, OP.add)
                lnq = wrk.tile([B_LOC, 1], f32, tag="lnq", name=f"lnq{it}")
                den = wrk.tile([B_LOC, 1], f32, tag="den", name=f"den{it}")
                if it == 0:
                    nc.scalar.activation(lnq[:], qpre[:], AF.Ln)
                    nc.vector.tensor_scalar_add(den[:], qpre[:], 1.0)
                else:
                    nc.scalar.activation(lnq[:], qpre[:], AF.Ln, scale=r2[:])
                    nc.vector.scalar_tensor_tensor(den[:], qpre[:], r2[:],
                                                   ones32[:], OP.mult, OP.add)
                nrm = wrk.tile([B_LOC, 1], f32, tag="nrm", name=f"nrm{it}")
                nc.scalar.activation(nrm[:], lnq[:], AF.Exp, scale=0.5)
                rden = wrk.tile([B_LOC, 1], f32, tag="rden", name=f"rden{it}")
                nc.vector.reciprocal(rden[:], den[:])
                scale = wrk.tile([B_LOC, 1], f32, tag="scl", name=f"scl{it}")
                nc.vector.tensor_tensor(scale[:], nrm[:], rden[:], OP.mult)
                if it == 0:
                    c_ap = scale  # rsum0 == 1 exactly
                else:
                    c_t = wrk.tile([B_LOC, 1], f32, tag="c", name=f"c{it}")
                    nc.vector.tensor_tensor(c_t[:], scale[:], rsum[:], OP.mult)
                    c_ap = c_t

                if it < ITERATIONS - 1:
                    # delta path (Pool + PE), parallel with the norm chain
                    ms = wrk.tile([B_LOC, O], f32, tag="ms", name=f"ms{it}")
                    # gpsimd cannot read PSUM: it0's S lives in SBUF, later
                    # iterations' S comes from psum -> use DVE there.
                    ms_eng = nc.gpsimd if it == 0 else nc.vector
                    ms_eng.tensor_tensor(ms[:], M2[:], s_ap, OP.mult)
                    ps_t = ps.tile([O, B_LOC], f32, tag="ps", name=f"ps_t{it}")
                    nc.tensor.transpose(ps_t[:], ms[:], ident[:B_LOC, :B_LOC])
                    tT = wrk.tile([O, B_LOC], f32, tag="tT", name=f"tT{it}")
                    nc.scalar.copy(tT[:], ps_t[:])
                    ps_d = ps.tile([B_LOC, N_CAPS], f32, tag="ps",
                                   name=f"ps_d{it}")
                    nc.tensor.matmul(ps_d[:], tT[:], wt_on[:],
                                     start=True, stop=True)
                    # join: logits += c * delta; softmax via deferred rsum
                    ex = wrk.tile([B_LOC, N_CAPS], f32, tag="ex", name=f"ex{it}")
                    ssum = wrk.tile([B_LOC, 1], f32, tag="ssum", name=f"ss{it}")
                    lg_new = wrk.tile([B_LOC, N_CAPS], f32, tag="lg",
                                      bufs=2, name=f"lg{it}")
                    if it == 0:
                        nc.scalar.activation(ex[:], ps_d[:], AF.Exp,
                                             scale=c_ap[:], accum_out=ssum[:])
                        nc.vector.tensor_scalar_mul(lg_new[:], ps_d[:], c_ap[:])
                    else:
                        nc.vector.scalar_tensor_tensor(lg_new[:], ps_d[:],
                                                       c_ap[:], lg[:],
                                                       OP.mult, OP.add)
                        nc.scalar.activation(ex[:], lg_new[:], AF.Exp,
                                             accum_out=ssum[:])
                    lg = lg_new
                    rsum = wrk.tile([B_LOC, 1], f32, tag="rsum", name=f"rs{it}")
                    nc.vector.reciprocal(rsum[:], ssum[:])
                    r2 = wrk.tile([B_LOC, 1], f32, tag="r2", name=f"r2{it}")
                    nc.vector.tensor_tensor(r2[:], rsum[:], rsum[:], OP.mult)
                    ps_ct = ps.tile([N_CAPS, B_LOC], f32, tag="ps",
                                    name=f"ps_ct{it}")
                    nc.tensor.transpose(ps_ct[:], ex[:], ident[:B_LOC, :B_LOC])
                    exT = wrk.tile([N_CAPS, B_LOC], f32, tag="exT",
                                   name=f"exT{it}")
                    nc.vector.tensor_copy(exT[:], ps_ct[:])
                else:
                    out_sb = wrk.tile([B_LOC, O], f32, tag="out_sb")
                    nc.vector.tensor_scalar_mul(out_sb[:], xs[:], c_ap[:])
                    nc.sync.dma_start(out_d[:], out_sb[:])

    nc.compile()
    return nc


def run_with_results(x: np.ndarray, caps_weights: np.ndarray, n_xq=2,
                     stage="full", **run_kwargs):
    """Run the SPMD kernel; returns (output (256,1,128), BassKernelResults)."""
    from concourse.bass_utils import run_bass_kernel_spmd

    key = f"nc{n_xq}_{stage}"
    if key not in _cache:
        _cache[key] = _build(n_xq, stage)
    nc = _cache[key]

    x = np.ascontiguousarray(x, dtype=np.float32)
    caps_weights = np.ascontiguousarray(caps_weights, dtype=np.float32)
    ones = np.ones((128, 1), dtype=np.float32)

    in_maps = []
    for c in range(N_CORES):
        in_maps.append({
            "x": np.ascontiguousarray(x[:, c * B_LOC:(c + 1) * B_LOC, :]),
            "caps_weights": caps_weights,
            "ones": ones,
        })
    res = run_bass_kernel_spmd(nc, in_maps, core_ids=list(range(N_CORES)),
                               **run_kwargs)
    out = np.concatenate([res.results[c]["out"] for c in range(N_CORES)], axis=0)
    return out.reshape(BATCH, 1, O), res


def kernel(x: np.ndarray, caps_weights: np.ndarray) -> np.ndarray:
    out, _ = run_with_results(x, caps_weights)
    return out


# revision 14
# speedup vs baseline: 1.1317x; 1.1317x over previous
"""Trainium2 Bass kernel for nn_Capsule: capsule routing head.

Math: the einsum 'nco,pbo->bno' factorizes as xp[b,n,o] = W[n,o] * X[b,o]
with W = caps_weights.sum(c) (64x128) and X = x.sum(p) (256x128), so the
kernel is a memory-bound reduction of x (151 MB total, 18.9 MB/core)
followed by a tiny per-batch routing loop.

Sharding: data-parallel over batch (dim 1 of x), 32 batch elements per
core; caps_weights replicated; no cross-core communication.

Per-core pipeline (v4):
  - x streams over 3 DMA queues with rate-proportional column shares:
    sync/scalar HWDGE rings sustain ~200 GB/s each, the gpsimd SWDGE
    queue ~100 GB/s (measured), so shares are ~40/40/20. Slabs complete
    sequentially every ~4.2us.
  - p-reduction: slabs 1,2 add into slab-0's buffer (accA) and 4,5 into
    slab-3's (accB) on DVE(h0)+Pool(h1) — in-stream adds run 2-3x slower
    than isolated (SBUF port contention with the DMA stream), so only 2
    adds per accumulator. PE reduces with an ALL-ONES (128,1) fp32r
    stationary: psum chunk c (1,512) accumulates 5 passes (accA, accB,
    raw slabs 6,7,8) at full fp32r rate (moving free 512).
  - slab 8's DMA is chunk-split; each stop-matmul is followed by a
    psum->SBUF copy (DVE/ACT alternating) and a per-chunk SBUF->DRAM
    write, so the flat->(b,o) rearrange (a pure reshape through DRAM;
    SBUF->SBUF cross-partition scatter is not expressible in DGE
    descriptors) overlaps the stream tail; one gather DMA lands X (32,128).
  - Routing in b-on-partitions layout; sqrt(q) = Exp(0.5*Ln(q)) keeps all
    ACT functions in one pinned table; the delta path (M2=X*X -> MS=M2*S
    -> PE transpose -> matmul) runs parallel to the norm chain; softmax
    normalization is deferred through rsum (folded into Ln via scale);
    softmax sums use DVE tensor_reduce (ACT accum_out costs an extra
    READ_ACCUMULATOR); NO tensor_tensor_reduce (crashes the exec unit:
    NRT_EXEC_UNIT_UNRECOVERABLE on this runtime).
"""

import numpy as np

# ---- problem constants (hardcoded per contract) ----
P_TOT = 1152
BATCH = 256
O = 128
N_CAPS = 64
CAPS_DIM = 16
ITERATIONS = 3
N_CORES = 8
B_LOC = BATCH // N_CORES          # 32 batch elements per core
PT = P_TOT // 128                 # 9 p-slabs
FLAT = B_LOC * O                  # 4096 flat (b,o) elements
CH = 512                          # psum chunk (max fp32 free per bank)
NCH = FLAT // CH                  # 8 chunks
HALF = FLAT // 2

_cache = {}


def _pin_act_table():
    """Force every ACT function onto the one table containing
    Exp+Ln+Square+Copy, so the kernel needs a single ACT_TABLE_LOAD."""
    import functools
    import concourse.hw_specs as hw_specs
    import concourse.bacc as bacc_mod

    if getattr(hw_specs.get_activation_tables, "_capsule_pinned", False):
        return
    orig = hw_specs.get_activation_tables

    @functools.cache
    def pinned(module_arch):
        tabs = orig(module_arch)
        keep = None
        for name, fns in tabs.items():
            names = {f.name for f in fns}
            if {"Exp", "Ln", "Square", "Copy", "Identity"} <= names:
                keep = name
                break
        if keep is None:
            return tabs
        return {n: (fns if n == keep else type(fns)()) for n, fns in tabs.items()}

    pinned._capsule_pinned = True
    hw_specs.get_activation_tables = pinned
    bacc_mod.get_activation_tables = pinned


def _build(n_xq=3, stage="full"):
    """n_xq: 2 = sync+scalar HWDGE rings; 3 = + gpsimd SWDGE queue."""
    _pin_act_table()
    import concourse.bacc as bacc
    import concourse.tile as tile
    import concourse.mybir as mybir
    from concourse.masks import make_identity

    f32 = mybir.dt.float32
    f32r = mybir.dt.float32r
    AF = mybir.ActivationFunctionType
    AX = mybir.AxisListType
    OP = mybir.AluOpType

    nc = bacc.Bacc(None, target_bir_lowering=False)

    x_in = nc.dram_tensor("x", [P_TOT, B_LOC, O], f32r, kind="ExternalInput")
    w_in = nc.dram_tensor("caps_weights", [N_CAPS, CAPS_DIM, O], f32,
                          kind="ExternalInput")
    ones_in = nc.dram_tensor("ones", [128, 1], f32r, kind="ExternalInput")
    out_d = nc.dram_tensor("out", [B_LOC, O], f32, kind="ExternalOutput")
    # DRAM bounce for the flat->(b,o) rearrange (pure reshape through DRAM;
    # SBUF->SBUF cross-partition scatter is not expressible in DGE
    # descriptors — verified wrong on HW)
    xsc = nc.dram_tensor("xscratch", [B_LOC, O], f32, kind="Internal")
    xsc_flat = xsc.rearrange("b o -> (b o)")

    xv = x_in.rearrange("(t p) b o -> t p (b o)", p=128)  # (9, 128, 4096)

    if n_xq == 2:
        # sync gets more columns: the scalar ring starts ~2.7us later
        # (ACT table load at its queue head)
        cuts = [0, 2112, FLAT]
    else:
        # ~42/38/20 rate-proportional shares (SWDGE sustains ~100 GB/s;
        # scalar starts later; slab 8 goes HWDGE-only so those two carry
        # a bit more overall)
        cuts = [0, 1728, 3264, FLAT]

    with tile.TileContext(nc) as tc:
        with (
            tc.tile_pool(name="xin", bufs=1) as xpool,
            tc.tile_pool(name="wrk", bufs=1) as wrk,
            tc.tile_pool(name="ps", bufs=8, space="PSUM") as ps,
        ):
            # ---- destination tiles for the stream ----
            accA = xpool.tile([128, FLAT], f32r, tag="accA")
            accB = xpool.tile([128, FLAT], f32r, tag="accB")
            slot_of = {1: 0, 2: 1, 4: 2, 5: 3, 6: 4, 7: 0, 8: 1}
            sl_tiles = [xpool.tile([128, FLAT], f32r, tag="slab", bufs=5,
                                   name=f"sl{i}") for i in range(5)]

            def dest(t):
                if t == 0:
                    return accA
                if t == 3:
                    return accB
                return sl_tiles[slot_of[t]]

            qengs = ([nc.sync, nc.scalar] if n_xq == 2
                     else [nc.sync, nc.scalar, nc.gpsimd])

            # Tile deps are PROGRAM-ORDER based: an add/matmul reading a slab
            # buffer binds to the most recent DMA into that buffer emitted
            # BEFORE it — DMA(t), add(t), and passes must be interleaved.
            def emit_dma(t):
                d = dest(t)
                if t == 8:
                    # the tail slab rides only the fast HWDGE rings
                    for c in range(NCH):
                        eng = qengs[c % 2]
                        eng.dma_start(d[:, c * CH:(c + 1) * CH],
                                      xv[8][:, c * CH:(c + 1) * CH])
                else:
                    for qi, eng in enumerate(qengs):
                        eng.dma_start(d[:, cuts[qi]:cuts[qi + 1]],
                                      xv[t][:, cuts[qi]:cuts[qi + 1]])

            def emit_add(acc, t):
                s = dest(t)
                nc.vector.tensor_tensor(acc[:, 0:HALF], acc[:, 0:HALF],
                                        s[:, 0:HALF], OP.add)
                nc.gpsimd.tensor_tensor(acc[:, HALF:FLAT], acc[:, HALF:FLAT],
                                        s[:, HALF:FLAT], OP.add)

            # ---- stream prologue: x first on every queue ----
            emit_dma(0)
            w_sb = wrk.tile([N_CAPS, CAPS_DIM * O], f32)
            nc.gpsimd.dma_start(w_sb[:], w_in.rearrange("n c o -> n (c o)"))
            ones_st = wrk.tile([128, 1], f32r)
            nc.gpsimd.dma_start(ones_st[:], ones_in[:])
            emit_dma(1)

            # ---- constants ----
            ident = wrk.tile([128, 128], f32)
            make_identity(nc, ident[:])
            ones32 = wrk.tile([B_LOC, 1], f32)
            nc.vector.memset(ones32[:], 1.0)
            unif = wrk.tile([N_CAPS, B_LOC], f32)
            nc.vector.memset(unif[:], 1.0 / N_CAPS)

            # ---- capsule weight prep (overlaps the x stream) ----
            t1 = wrk.tile([N_CAPS, 8 * O], f32)
            nc.vector.tensor_tensor(t1[:], w_sb[:, :8 * O], w_sb[:, 8 * O:], OP.add)
            t2 = wrk.tile([N_CAPS, 4 * O], f32)
            nc.vector.tensor_tensor(t2[:], t1[:, :4 * O], t1[:, 4 * O:], OP.add)
            t3 = wrk.tile([N_CAPS, 2 * O], f32)
            nc.vector.tensor_tensor(t3[:], t2[:, :2 * O], t2[:, 2 * O:], OP.add)
            w_no = wrk.tile([N_CAPS, O], f32)          # W[n,o]
            nc.vector.tensor_tensor(w_no[:], t3[:, :O], t3[:, O:], OP.add)

            ps_wt = ps.tile([O, N_CAPS], f32, tag="ps")
            nc.tensor.transpose(ps_wt[:], w_no[:], ident[:N_CAPS, :N_CAPS])
            wt_on = wrk.tile([O, N_CAPS], f32)          # W^T[o,n]
            nc.vector.tensor_copy(wt_on[:], ps_wt[:])
            ps_s0 = ps.tile([B_LOC, O], f32, tag="ps", name="ps_s0")
            nc.tensor.matmul(ps_s0[:], unif[:], w_no[:], start=True, stop=True)
            s0_sb = wrk.tile([B_LOC, O], f32)           # S0 = (1/64) sum_n W
            nc.vector.tensor_copy(s0_sb[:], ps_s0[:])

            # ---- stream body: 5 PE passes over accA/accB/slabs 6,7,8 ----
            ps_ch = [ps.tile([1, CH], f32, tag="ps", name=f"ps_ch{c}")
                     for c in range(NCH)]

            def emit_pass(src, start, stop):
                for c in range(NCH):
                    cs, ce = c * CH, (c + 1) * CH
                    nc.tensor.matmul(ps_ch[c][:], ones_st[:], src[:, cs:ce],
                                     start=start, stop=stop,
                                     skip_group_check=True)

            emit_add(accA, 1)
            emit_dma(2)
            emit_add(accA, 2)
            emit_pass(accA, True, False)        # A-pass
            emit_dma(3)
            emit_dma(4)
            emit_add(accB, 4)
            emit_dma(5)
            emit_add(accB, 5)
            emit_pass(accB, False, False)       # B-pass
            emit_dma(6)
            emit_pass(dest(6), False, False)
            emit_dma(7)
            emit_pass(dest(7), False, False)
            emit_dma(8)
            # stop-pass chunk-by-chunk; copy + DRAM write ride each chunk
            xflat = wrk.tile([1, FLAT], f32)
            s8 = dest(8)
            for c in range(NCH):
                cs, ce = c * CH, (c + 1) * CH
                nc.tensor.matmul(ps_ch[c][:], ones_st[:], s8[:, cs:ce],
                                 start=False, stop=True, skip_group_check=True)
                if c % 2 == 0:
                    nc.vector.tensor_copy(xflat[:, cs:ce], ps_ch[c][:])
                else:
                    nc.scalar.copy(xflat[:, cs:ce], ps_ch[c][:])
                qengs[c % 2].dma_start(xsc_flat[None, cs:ce], xflat[:, cs:ce])

            # ---- gather X (32,128) from the bounce ----
            x32 = wrk.tile([B_LOC, O], f32)
            nc.sync.dma_start(x32[:], xsc[:])

            if stage == "x32":
                o_dbg = wrk.tile([B_LOC, O], f32)
                nc.vector.tensor_scalar_add(o_dbg[:], x32[:], 0.0)
                nc.sync.dma_start(out_d[:], o_dbg[:])

            # ---- routing (b on partitions) ----
            M2 = wrk.tile([B_LOC, O], f32)
            nc.gpsimd.tensor_tensor(M2[:], x32[:], x32[:], OP.mult)

            iters = ITERATIONS if stage == "full" else 0
            sq = wrk.tile([B_LOC, O], f32)
            lg = None
            rsum = None
            r2 = None
            exT = None
            for it in range(iters):
                if it == 0:
                    s_ap = s0_sb[:]
                else:
                    ps_s = ps.tile([B_LOC, O], f32, tag="ps", name=f"ps_s{it}")
                    nc.tensor.matmul(ps_s[:], exT[:], w_no[:],
                                     start=True, stop=True)
                    s_ap = ps_s[:]
                # norm chain (DVE + ACT)
                xs = wrk.tile([B_LOC, O], f32, tag="xs", name=f"xs{it}")
                nc.vector.tensor_tensor(xs[:], x32[:], s_ap, OP.mult)
                qpre = wrk.tile([B_LOC, 1], f32, tag="qpre", name=f"qpre{it}")
                # (no tensor_tensor_reduce: it crashes the exec unit on this
                #  runtime -> NRT_EXEC_UNIT_UNRECOVERABLE)
                nc.vector.tensor_tensor(sq[:], xs[:], xs[:], OP.mult)
                nc.vector.tensor_reduce(qpre[:], sq[:], AX.X, OP.add)
                lnq = wrk.tile([B_LOC, 1], f32, tag="lnq", name=f"lnq{it}")
                den = wrk.tile([B_LOC, 1], f32, tag="den", name=f"den{it}")
                if it == 0:
                    nc.scalar.activation(lnq[:], qpre[:], AF.Ln)
                    nc.vector.tensor_scalar_add(den[:], qpre[:], 1.0)
                else:
                    nc.scalar.activation(lnq[:], qpre[:], AF.Ln, scale=r2[:])
                    nc.vector.scalar_tensor_tensor(den[:], qpre[:], r2[:],
                                                   ones32[:], OP.mult, OP.add)
                nrm = wrk.tile([B_LOC, 1], f32, tag="nrm", name=f"nrm{it}")
                nc.scalar.activation(nrm[:], lnq[:], AF.Exp, scale=0.5)
                rden = wrk.tile([B_LOC, 1], f32, tag="rden", name=f"rden{it}")
                nc.vector.reciprocal(rden[:], den[:])
                scale = wrk.tile([B_LOC, 1], f32, tag="scl", name=f"scl{it}")
                nc.vector.tensor_tensor(scale[:], nrm[:], rden[:], OP.mult)
                if it == 0:
                    c_ap = scale  # rsum0 == 1 exactly
                else:
                    c_t = wrk.tile([B_LOC, 1], f32, tag="c", name=f"c{it}")
                    nc.vector.tensor_tensor(c_t[:], scale[:], rsum[:], OP.mult)
                    c_ap = c_t

                if it < ITERATIONS - 1:
                    # delta path (PE-heavy), parallel with the norm chain
                    ms = wrk.tile([B_LOC, O], f32, tag="ms", name=f"ms{it}")
                    ms_eng = nc.gpsimd if it == 0 else nc.vector
                    ms_eng.tensor_tensor(ms[:], M2[:], s_ap, OP.mult)
                    ps_t = ps.tile([O, B_LOC], f32, tag="ps", name=f"ps_t{it}")
                    nc.tensor.transpose(ps_t[:], ms[:], ident[:B_LOC, :B_LOC])
                    tT = wrk.tile([O, B_LOC], f32, tag="tT", name=f"tT{it}")
                    nc.scalar.copy(tT[:], ps_t[:])
                    ps_d = ps.tile([B_LOC, N_CAPS], f32, tag="ps",
                                   name=f"ps_d{it}")
                    nc.tensor.matmul(ps_d[:], tT[:], wt_on[:],
                                     start=True, stop=True)
                    # join: logits += c * delta; softmax via deferred rsum
                    ex = wrk.tile([B_LOC, N_CAPS], f32, tag="ex", name=f"ex{it}")
                    lg_new = wrk.tile([B_LOC, N_CAPS], f32, tag="lg",
                                      bufs=2, name=f"lg{it}")
                    if it == 0:
                        nc.scalar.activation(ex[:], ps_d[:], AF.Exp,
                                             scale=c_ap[:])
                        nc.vector.tensor_scalar_mul(lg_new[:], ps_d[:], c_ap[:])
                    else:
                        nc.vector.scalar_tensor_tensor(lg_new[:], ps_d[:],
                                                       c_ap[:], lg[:],
                                                       OP.mult, OP.add)
                        nc.scalar.activation(ex[:], lg_new[:], AF.Exp)
                    lg = lg_new
                    # softmax sum on DVE (no ACT READ_ACCUMULATOR; runs
                    # parallel with the ex transpose on PE)
                    ssum = wrk.tile([B_LOC, 1], f32, tag="ssum", name=f"ss{it}")
                    nc.vector.tensor_reduce(ssum[:], ex[:], AX.X, OP.add)
                    rsum = wrk.tile([B_LOC, 1], f32, tag="rsum", name=f"rs{it}")
                    nc.vector.reciprocal(rsum[:], ssum[:])
                    r2 = wrk.tile([B_LOC, 1], f32, tag="r2", name=f"r2{it}")
                    nc.vector.tensor_tensor(r2[:], rsum[:], rsum[:], OP.mult)
                    ps_ct = ps.tile([N_CAPS, B_LOC], f32, tag="ps",
                                    name=f"ps_ct{it}")
                    nc.tensor.transpose(ps_ct[:], ex[:], ident[:B_LOC, :B_LOC])
                    exT = wrk.tile([N_CAPS, B_LOC], f32, tag="exT",
                                   name=f"exT{it}")
                    nc.scalar.copy(exT[:], ps_ct[:])
                else:
                    out_sb = wrk.tile([B_LOC, O], f32, tag="out_sb")
                    nc.vector.tensor_scalar_mul(out_sb[:], xs[:], c_ap[:])
                    nc.sync.dma_start(out_d[:], out_sb[:])

    nc.compile()
    return nc


def run_with_results(x: np.ndarray, caps_weights: np.ndarray, n_xq=3,
                     stage="full", **run_kwargs):
    """Run the SPMD kernel; returns (output (256,1,128), BassKernelResults)."""
    from concourse.bass_utils import run_bass_kernel_spmd

    key = f"nc{n_xq}_{stage}"
    if key not in _cache:
        _cache[key] = _build(n_xq, stage)
    nc = _cache[key]

    x = np.ascontiguousarray(x, dtype=np.float32)
    caps_weights = np.ascontiguousarray(caps_weights, dtype=np.float32)
    ones = np.ones((128, 1), dtype=np.float32)

    in_maps = []
    for c in range(N_CORES):
        in_maps.append({
            "x": np.ascontiguousarray(x[:, c * B_LOC:(c + 1) * B_LOC, :]),
            "caps_weights": caps_weights,
            "ones": ones,
        })
    res = run_bass_kernel_spmd(nc, in_maps, core_ids=list(range(N_CORES)),
                               **run_kwargs)
    out = np.concatenate([res.results[c]["out"] for c in range(N_CORES)], axis=0)
    return out.reshape(BATCH, 1, O), res


def kernel(x: np.ndarray, caps_weights: np.ndarray) -> np.ndarray:
    out, _ = run_with_results(x, caps_weights)
    return out


# revision 15
# speedup vs baseline: 1.1819x; 1.0443x over previous
"""Trainium2 Bass kernel for nn_Capsule: capsule routing head.

Math: the einsum 'nco,pbo->bno' factorizes as xp[b,n,o] = W[n,o] * X[b,o]
with W = caps_weights.sum(c) (64x128) and X = x.sum(p) (256x128), so the
kernel is a memory-bound reduction of x (151 MB total, 18.9 MB/core)
followed by a tiny per-batch routing loop.

Sharding: data-parallel over batch (dim 1 of x), 32 batch elements per
core; caps_weights replicated; no cross-core communication.

Per-core pipeline (v4):
  - x streams over 3 DMA queues with rate-proportional column shares:
    sync/scalar HWDGE rings sustain ~200 GB/s each, the gpsimd SWDGE
    queue ~100 GB/s (measured), so shares are ~40/40/20. Slabs complete
    sequentially every ~4.2us.
  - p-reduction: slabs 1,2 add into slab-0's buffer (accA) and 4,5 into
    slab-3's (accB) on DVE(h0)+Pool(h1) — in-stream adds run 2-3x slower
    than isolated (SBUF port contention with the DMA stream), so only 2
    adds per accumulator. PE reduces with an ALL-ONES (128,1) fp32r
    stationary: psum chunk c (1,512) accumulates 5 passes (accA, accB,
    raw slabs 6,7,8) at full fp32r rate (moving free 512).
  - slab 8's DMA is chunk-split; each stop-matmul is followed by a
    psum->SBUF copy (DVE/ACT alternating) and a per-chunk SBUF->DRAM
    write, so the flat->(b,o) rearrange (a pure reshape through DRAM;
    SBUF->SBUF cross-partition scatter is not expressible in DGE
    descriptors) overlaps the stream tail; one gather DMA lands X (32,128).
  - Routing in b-on-partitions layout; sqrt(q) = Exp(0.5*Ln(q)) keeps all
    ACT functions in one pinned table; the delta path (M2=X*X -> MS=M2*S
    -> PE transpose -> matmul) runs parallel to the norm chain; softmax
    normalization is deferred through rsum (folded into Ln via scale);
    softmax sums use DVE tensor_reduce (ACT accum_out costs an extra
    READ_ACCUMULATOR); NO tensor_tensor_reduce (crashes the exec unit:
    NRT_EXEC_UNIT_UNRECOVERABLE on this runtime).
"""

import numpy as np

# ---- problem constants (hardcoded per contract) ----
P_TOT = 1152
BATCH = 256
O = 128
N_CAPS = 64
CAPS_DIM = 16
ITERATIONS = 3
N_CORES = 8
B_LOC = BATCH // N_CORES          # 32 batch elements per core
PT = P_TOT // 128                 # 9 p-slabs
FLAT = B_LOC * O                  # 4096 flat (b,o) elements
CH = 512                          # psum chunk (max fp32 free per bank)
NCH = FLAT // CH                  # 8 chunks
HALF = FLAT // 2

_cache = {}


def _pin_act_table():
    """Force every ACT function onto the one table containing
    Exp+Ln+Square+Copy, so the kernel needs a single ACT_TABLE_LOAD."""
    import functools
    import concourse.hw_specs as hw_specs
    import concourse.bacc as bacc_mod

    if getattr(hw_specs.get_activation_tables, "_capsule_pinned", False):
        return
    orig = hw_specs.get_activation_tables

    @functools.cache
    def pinned(module_arch):
        tabs = orig(module_arch)
        keep = None
        for name, fns in tabs.items():
            names = {f.name for f in fns}
            if {"Exp", "Ln", "Square", "Copy", "Identity"} <= names:
                keep = name
                break
        if keep is None:
            return tabs
        return {n: (fns if n == keep else type(fns)()) for n, fns in tabs.items()}

    pinned._capsule_pinned = True
    hw_specs.get_activation_tables = pinned
    bacc_mod.get_activation_tables = pinned


def _build(n_xq=2, stage="full"):
    """n_xq: 2 = sync+scalar HWDGE rings; 3 = + gpsimd SWDGE queue."""
    _pin_act_table()
    import concourse.bacc as bacc
    import concourse.tile as tile
    import concourse.mybir as mybir
    from concourse.masks import make_identity

    f32 = mybir.dt.float32
    f32r = mybir.dt.float32r
    AF = mybir.ActivationFunctionType
    AX = mybir.AxisListType
    OP = mybir.AluOpType

    nc = bacc.Bacc(None, target_bir_lowering=False)

    x_in = nc.dram_tensor("x", [P_TOT, B_LOC, O], f32r, kind="ExternalInput")
    w_in = nc.dram_tensor("caps_weights", [N_CAPS, CAPS_DIM, O], f32,
                          kind="ExternalInput")
    ones_in = nc.dram_tensor("ones", [128, 1], f32r, kind="ExternalInput")
    out_d = nc.dram_tensor("out", [B_LOC, O], f32, kind="ExternalOutput")
    # DRAM bounce for the flat->(b,o) rearrange (pure reshape through DRAM;
    # SBUF->SBUF cross-partition scatter is not expressible in DGE
    # descriptors — verified wrong on HW)
    xsc = nc.dram_tensor("xscratch", [B_LOC, O], f32, kind="Internal")
    xsc_flat = xsc.rearrange("b o -> (b o)")

    xv = x_in.rearrange("(t p) b o -> t p (b o)", p=128)  # (9, 128, 4096)

    if n_xq == 2:
        # sync gets more columns: the scalar ring starts ~2.7us later
        # (ACT table load at its queue head)
        cuts = [0, 2112, FLAT]
    else:
        # ~42/38/20 rate-proportional shares (SWDGE sustains ~100 GB/s;
        # scalar starts later; slab 8 goes HWDGE-only so those two carry
        # a bit more overall)
        cuts = [0, 1728, 3264, FLAT]

    with tile.TileContext(nc) as tc:
        with (
            tc.tile_pool(name="xin", bufs=1) as xpool,
            tc.tile_pool(name="wrk", bufs=1) as wrk,
            tc.tile_pool(name="ps", bufs=8, space="PSUM") as ps,
        ):
            # ---- destination tiles for the stream ----
            accA = xpool.tile([128, FLAT], f32r, tag="accA")
            accB = xpool.tile([128, FLAT], f32r, tag="accB")
            slot_of = {1: 0, 2: 1, 4: 2, 5: 3, 6: 4, 7: 0, 8: 1}
            sl_tiles = [xpool.tile([128, FLAT], f32r, tag="slab", bufs=5,
                                   name=f"sl{i}") for i in range(5)]

            def dest(t):
                if t == 0:
                    return accA
                if t == 3:
                    return accB
                return sl_tiles[slot_of[t]]

            qengs = ([nc.sync, nc.scalar] if n_xq == 2
                     else [nc.sync, nc.scalar, nc.gpsimd])

            # Tile deps are PROGRAM-ORDER based: an add/matmul reading a slab
            # buffer binds to the most recent DMA into that buffer emitted
            # BEFORE it — DMA(t), add(t), and passes must be interleaved.
            def emit_dma(t):
                d = dest(t)
                if t == 8:
                    # the tail slab rides only the fast HWDGE rings
                    for c in range(NCH):
                        eng = qengs[c % 2]
                        eng.dma_start(d[:, c * CH:(c + 1) * CH],
                                      xv[8][:, c * CH:(c + 1) * CH])
                else:
                    for qi, eng in enumerate(qengs):
                        eng.dma_start(d[:, cuts[qi]:cuts[qi + 1]],
                                      xv[t][:, cuts[qi]:cuts[qi + 1]])

            def emit_add(acc, t):
                s = dest(t)
                nc.vector.tensor_tensor(acc[:, 0:HALF], acc[:, 0:HALF],
                                        s[:, 0:HALF], OP.add)
                nc.gpsimd.tensor_tensor(acc[:, HALF:FLAT], acc[:, HALF:FLAT],
                                        s[:, HALF:FLAT], OP.add)

            # ---- stream prologue: x first on every queue ----
            emit_dma(0)
            w_sb = wrk.tile([N_CAPS, CAPS_DIM * O], f32)
            nc.gpsimd.dma_start(w_sb[:], w_in.rearrange("n c o -> n (c o)"))
            ones_st = wrk.tile([128, 1], f32r)
            nc.gpsimd.dma_start(ones_st[:], ones_in[:])
            emit_dma(1)

            # ---- constants ----
            ident = wrk.tile([128, 128], f32)
            make_identity(nc, ident[:])
            ones32 = wrk.tile([B_LOC, 1], f32)
            nc.vector.memset(ones32[:], 1.0)
            unif = wrk.tile([N_CAPS, B_LOC], f32)
            nc.vector.memset(unif[:], 1.0 / N_CAPS)

            # ---- capsule weight prep (overlaps the x stream) ----
            t1 = wrk.tile([N_CAPS, 8 * O], f32)
            nc.vector.tensor_tensor(t1[:], w_sb[:, :8 * O], w_sb[:, 8 * O:], OP.add)
            t2 = wrk.tile([N_CAPS, 4 * O], f32)
            nc.vector.tensor_tensor(t2[:], t1[:, :4 * O], t1[:, 4 * O:], OP.add)
            t3 = wrk.tile([N_CAPS, 2 * O], f32)
            nc.vector.tensor_tensor(t3[:], t2[:, :2 * O], t2[:, 2 * O:], OP.add)
            w_no = wrk.tile([N_CAPS, O], f32)          # W[n,o]
            nc.vector.tensor_tensor(w_no[:], t3[:, :O], t3[:, O:], OP.add)

            ps_wt = ps.tile([O, N_CAPS], f32, tag="ps")
            nc.tensor.transpose(ps_wt[:], w_no[:], ident[:N_CAPS, :N_CAPS])
            wt_on = wrk.tile([O, N_CAPS], f32)          # W^T[o,n]
            nc.vector.tensor_copy(wt_on[:], ps_wt[:])
            ps_s0 = ps.tile([B_LOC, O], f32, tag="ps", name="ps_s0")
            nc.tensor.matmul(ps_s0[:], unif[:], w_no[:], start=True, stop=True)
            s0_sb = wrk.tile([B_LOC, O], f32)           # S0 = (1/64) sum_n W
            nc.vector.tensor_copy(s0_sb[:], ps_s0[:])
            bf16 = mybir.dt.bfloat16
            w_no_bf = wrk.tile([N_CAPS, O], bf16)       # bf16 W for S-matmuls
            nc.vector.tensor_copy(w_no_bf[:], w_no[:])

            # ---- stream body: 5 PE passes over accA/accB/slabs 6,7,8 ----
            ps_ch = [ps.tile([1, CH], f32, tag="ps", name=f"ps_ch{c}")
                     for c in range(NCH)]

            def emit_pass(src, start, stop):
                for c in range(NCH):
                    cs, ce = c * CH, (c + 1) * CH
                    nc.tensor.matmul(ps_ch[c][:], ones_st[:], src[:, cs:ce],
                                     start=start, stop=stop,
                                     skip_group_check=True)

            emit_add(accA, 1)
            emit_dma(2)
            emit_add(accA, 2)
            emit_pass(accA, True, False)        # A-pass
            emit_dma(3)
            emit_dma(4)
            emit_add(accB, 4)
            emit_dma(5)
            emit_add(accB, 5)
            emit_pass(accB, False, False)       # B-pass
            emit_dma(6)
            emit_pass(dest(6), False, False)
            emit_dma(7)
            emit_pass(dest(7), False, False)
            emit_dma(8)
            # stop-pass chunk-by-chunk; copy + DRAM write ride each chunk
            xflat = wrk.tile([1, FLAT], f32)
            s8 = dest(8)
            for c in range(NCH):
                cs, ce = c * CH, (c + 1) * CH
                nc.tensor.matmul(ps_ch[c][:], ones_st[:], s8[:, cs:ce],
                                 start=False, stop=True, skip_group_check=True)
                mid = cs + CH // 2
                nc.vector.tensor_copy(xflat[:, cs:mid], ps_ch[c][:, 0:CH // 2])
                nc.scalar.copy(xflat[:, mid:ce], ps_ch[c][:, CH // 2:CH])
                qengs[c % 2].dma_start(xsc_flat[None, cs:ce], xflat[:, cs:ce])

            # ---- gather X (32,128) from the bounce ----
            x32 = wrk.tile([B_LOC, O], f32)
            nc.sync.dma_start(x32[:], xsc[:])

            if stage == "x32":
                o_dbg = wrk.tile([B_LOC, O], f32)
                nc.vector.tensor_scalar_add(o_dbg[:], x32[:], 0.0)
                nc.sync.dma_start(out_d[:], o_dbg[:])

            # ---- routing (b on partitions) ----
            iters = ITERATIONS if stage == "full" else 0
            sq = wrk.tile([B_LOC, O], f32)
            lg = None
            rsum = None
            r2 = None
            exT = None
            for it in range(iters):
                if it == 0:
                    s_ap = s0_sb[:]
                else:
                    ps_s = ps.tile([B_LOC, O], f32, tag="ps", name=f"ps_s{it}")
                    nc.tensor.matmul(ps_s[:], exT[:], w_no_bf[:],
                                     start=True, stop=True)
                    s_ap = ps_s[:]
                # norm chain (DVE + ACT)
                xs = wrk.tile([B_LOC, O], f32, tag="xs", name=f"xs{it}")
                nc.vector.tensor_tensor(xs[:], x32[:], s_ap, OP.mult)
                qpre = wrk.tile([B_LOC, 1], f32, tag="qpre", name=f"qpre{it}")
                # fused square+sum via STT accum (tensor_tensor_reduce crashes
                # the exec unit on this runtime -> NRT_EXEC_UNIT_UNRECOVERABLE)
                nc.vector.scalar_tensor_tensor(sq[:], xs[:], 1.0, xs[:],
                                               OP.bypass, OP.mult,
                                               accum_out=qpre[:])
                lnq = wrk.tile([B_LOC, 1], f32, tag="lnq", name=f"lnq{it}")
                den = wrk.tile([B_LOC, 1], f32, tag="den", name=f"den{it}")
                if it == 0:
                    nc.scalar.activation(lnq[:], qpre[:], AF.Ln)
                    nc.vector.tensor_scalar_add(den[:], qpre[:], 1.0)
                else:
                    nc.scalar.activation(lnq[:], qpre[:], AF.Ln, scale=r2[:])
                    nc.vector.scalar_tensor_tensor(den[:], qpre[:], r2[:],
                                                   ones32[:], OP.mult, OP.add)
                nrm = wrk.tile([B_LOC, 1], f32, tag="nrm", name=f"nrm{it}")
                nc.scalar.activation(nrm[:], lnq[:], AF.Exp, scale=0.5)
                rden = wrk.tile([B_LOC, 1], f32, tag="rden", name=f"rden{it}")
                nc.vector.reciprocal(rden[:], den[:])
                scale = wrk.tile([B_LOC, 1], f32, tag="scl", name=f"scl{it}")
                nc.vector.tensor_tensor(scale[:], nrm[:], rden[:], OP.mult)
                if it == 0:
                    c_ap = scale  # rsum0 == 1 exactly
                else:
                    c_t = wrk.tile([B_LOC, 1], f32, tag="c", name=f"c{it}")
                    nc.vector.tensor_tensor(c_t[:], scale[:], rsum[:], OP.mult)
                    c_ap = c_t

                if it < ITERATIONS - 1:
                    # delta path (PE-heavy), parallel with the norm chain
                    # m = u*X = xs*X (both SBUF -> Pool engine, off the
                    # DVE critical chain)
                    ms = wrk.tile([B_LOC, O], f32, tag="ms", name=f"ms{it}")
                    nc.gpsimd.tensor_tensor(ms[:], xs[:], x32[:], OP.mult)
                    ps_t = ps.tile([O, B_LOC], f32, tag="ps", name=f"ps_t{it}")
                    nc.tensor.transpose(ps_t[:], ms[:], ident[:B_LOC, :B_LOC])
                    tT = wrk.tile([O, B_LOC], f32, tag="tT", name=f"tT{it}")
                    nc.scalar.copy(tT[:], ps_t[:])
                    ps_d = ps.tile([B_LOC, N_CAPS], f32, tag="ps",
                                   name=f"ps_d{it}")
                    nc.tensor.matmul(ps_d[:], tT[:], wt_on[:],
                                     start=True, stop=True)
                    # join: logits += c * delta; softmax via deferred rsum
                    ex = wrk.tile([B_LOC, N_CAPS], f32, tag="ex", name=f"ex{it}")
                    lg_new = wrk.tile([B_LOC, N_CAPS], f32, tag="lg",
                                      bufs=2, name=f"lg{it}")
                    if it == 0:
                        nc.scalar.activation(ex[:], ps_d[:], AF.Exp,
                                             scale=c_ap[:])
                        nc.vector.tensor_scalar_mul(lg_new[:], ps_d[:], c_ap[:])
                    else:
                        nc.vector.scalar_tensor_tensor(lg_new[:], ps_d[:],
                                                       c_ap[:], lg[:],
                                                       OP.mult, OP.add)
                        nc.scalar.activation(ex[:], lg_new[:], AF.Exp)
                    lg = lg_new
                    # softmax sum on DVE (no ACT READ_ACCUMULATOR; runs
                    # parallel with the ex transpose on PE)
                    ssum = wrk.tile([B_LOC, 1], f32, tag="ssum", name=f"ss{it}")
                    nc.vector.tensor_reduce(ssum[:], ex[:], AX.X, OP.add)
                    rsum = wrk.tile([B_LOC, 1], f32, tag="rsum", name=f"rs{it}")
                    nc.vector.reciprocal(rsum[:], ssum[:])
                    r2 = wrk.tile([B_LOC, 1], f32, tag="r2", name=f"r2{it}")
                    nc.vector.tensor_tensor(r2[:], rsum[:], rsum[:], OP.mult)
                    ps_ct = ps.tile([N_CAPS, B_LOC], f32, tag="ps",
                                    name=f"ps_ct{it}")
                    nc.tensor.transpose(ps_ct[:], ex[:], ident[:B_LOC, :B_LOC])
                    exT = wrk.tile([N_CAPS, B_LOC], bf16, tag="exT",
                                   name=f"exT{it}")
                    nc.scalar.copy(exT[:], ps_ct[:])
                else:
                    out_sb = wrk.tile([B_LOC, O], f32, tag="out_sb")
                    nc.vector.tensor_scalar_mul(out_sb[:], xs[:], c_ap[:])
                    nc.sync.dma_start(out_d[:], out_sb[:])

    nc.compile()
    return nc


def run_with_results(x: np.ndarray, caps_weights: np.ndarray, n_xq=2,
                     stage="full", **run_kwargs):
    """Run the SPMD kernel; returns (output (256,1,128), BassKernelResults)."""
    from concourse.bass_utils import run_bass_kernel_spmd

    key = f"nc{n_xq}_{stage}"
    if key not in _cache:
        _cache[key] = _build(n_xq, stage)
    nc = _cache[key]

    x = np.ascontiguousarray(x, dtype=np.float32)
    caps_weights = np.ascontiguousarray(caps_weights, dtype=np.float32)
    ones = np.ones((128, 1), dtype=np.float32)

    in_maps = []
    for c in range(N_CORES):
        in_maps.append({
            "x": np.ascontiguousarray(x[:, c * B_LOC:(c + 1) * B_LOC, :]),
            "caps_weights": caps_weights,
            "ones": ones,
        })
    res = run_bass_kernel_spmd(nc, in_maps, core_ids=list(range(N_CORES)),
                               **run_kwargs)
    out = np.concatenate([res.results[c]["out"] for c in range(N_CORES)], axis=0)
    return out.reshape(BATCH, 1, O), res


def kernel(x: np.ndarray, caps_weights: np.ndarray) -> np.ndarray:
    out, _ = run_with_results(x, caps_weights)
    return out
